# revision 15
# baseline (speedup 1.0000x reference)
"""AttentionCritic Trainium2 kernel (v2 — PE block-diag attention).

Problem (hardcoded): A=8 agents, B=8192 batch, S=128 state, ADIM=16 act,
H=512 hid, HEADS=4, D=128. 8 NeuronCores, batch-sharded (1024 batch/core).

Pipeline per core (b = 1024 local batch, chunks of 128):
  Phase A (chunk-outer, agent-inner):
    LN(states), LN([states|actions]) in fp32, normalized out in fp16;
    PE-transpose normalized inputs; fp16 matmuls for s_enc / sa_enc
    (batch-major out via activation-as-stationary); PE-transpose sa_enc;
    keys/vals/sel projections (all heads at once).
    Attention middle:
      logits via fused DVE tensor_tensor_reduce (one op per (k,i,j));
      exp/softmax-normalize into w fp16 [b,(j,k,i)];
      w and vals bounce through DRAM into a "fat" layout [(b16,j), ...];
      per (head, 16-batch group) a block-diagonal 128x128 stationary
      (mask * broadcast-w) turns the j-sum into a PE matmul;
      attended values evicted fat and DMA-scattered to DRAM batch-major.
  Phase B (agent-outer, 512-batch superchunks):
    LN(cin) stats in batch-major, normalize, PE-transpose, critic matmuls
    (h1 feature-major so bc1 folds into the Prelu eviction), all_qs out.
  Host: shard/unshard, fp16 weight casts, argmax-gather of all_qs.
"""
import sys

sys.path.insert(0, "/opt/trn_rl_repo")
from contextlib import ExitStack

import numpy as np

import concourse.bass as bass
import concourse.bacc as bacc
import concourse.mybir as mybir
from concourse import tile
from concourse.bass_utils import run_bass_kernel_spmd
from concourse.masks import make_identity

A, B, S, ADIM = 8, 8192, 128, 16
H, HEADS = 512, 4
D = H // HEADS
EPS = 1e-5
NCORES = 8
BL = B // NCORES          # local batch per core = 1024
P = 128                   # partition size
NCH = BL // P             # chunks per core = 8
NSC = BL // 512           # 512-batch superchunks = 2
SA = S + ADIM             # 144

f32 = mybir.dt.float32
f16 = mybir.dt.float16
FP = mybir.AluOpType
AF = mybir.ActivationFunctionType
AX = mybir.AxisListType
LRELU_SLOPE = 0.01
INV_SQRT_D = 1.0 / float(np.sqrt(D))


def _bc(ap, axis, n):
    """Insert a broadcast (step 0, count n) free dim at position `axis`
    (0 = first free dim)."""
    dims = [list(d) for d in ap.ap]
    dims.insert(1 + axis, [0, n])
    return bass.AP(ap.tensor, ap.offset, dims)


def _bc_front(ap, n):
    """Prepend a broadcast dim (for DRAM->SBUF partition replication)."""
    dims = [[0, n]] + [list(d) for d in ap.ap]
    return bass.AP(ap.tensor, ap.offset, dims)


def _patch_act_tables():
    """Restrict bacc's activation-table choices to the one set that covers
    every func we use (exp, ln, square, parametric_relu, identity, copy) so
    no ACT_TABLE_LOAD churn happens mid-kernel."""
    from concourse import hw_specs
    import concourse.bacc as _bacc

    orig = hw_specs.get_activation_tables

    def only_combined(arch):
        t = orig(arch)
        if "natural_log_exp_and_others" not in t:
            return t
        return {
            k: (v if k == "natural_log_exp_and_others" else set())
            for k, v in t.items()
        }

    only_combined.__wrapped__ = orig
    hw_specs.get_activation_tables = only_combined
    _bacc.get_activation_tables = only_combined


import os
_SKIP = set(os.environ.get("KSKIP", "").split(","))


def build(nonzero_bias):
    if "tables" not in _SKIP:
        _patch_act_tables()
    nc = bacc.Bacc("TRN2", target_bir_lowering=False, debug=False)

    # ---- DRAM I/O ----
    st_d = nc.dram_tensor("states", [A, BL, S], f32, kind="ExternalInput")
    ac_d = nc.dram_tensor("actions", [A, BL, ADIM], f32, kind="ExternalInput")
    ws_s_d = nc.dram_tensor("ws_s", [A, S, H], f16, kind="ExternalInput")
    ws_sa_d = nc.dram_tensor("ws_sa", [A, SA, H], f16, kind="ExternalInput")
    wk_d = nc.dram_tensor("wk", [H, H], f16, kind="ExternalInput")   # [h, (head,d)]
    wv_d = nc.dram_tensor("wv", [H, H], f16, kind="ExternalInput")
    wsel_d = nc.dram_tensor("wsel", [H, H], f16, kind="ExternalInput")
    wc1_d = nc.dram_tensor("wc1", [A, 2 * H, H], f16, kind="ExternalInput")
    wc2_d = nc.dram_tensor("wc2", [A, H, ADIM], f16, kind="ExternalInput")
    bs_s_d = nc.dram_tensor("bs_s", [A, H], f16, kind="ExternalInput")
    bs_sa_d = nc.dram_tensor("bs_sa", [A, H], f16, kind="ExternalInput")
    bsel_d = nc.dram_tensor("bsel", [H], f16, kind="ExternalInput")
    bc1_d = nc.dram_tensor("bc1", [A, H], f32, kind="ExternalInput")
    bc2_d = nc.dram_tensor("bc2", [A, ADIM], f32, kind="ExternalInput")

    allqs_d = nc.dram_tensor("allqs", [A, ADIM, BL], f32, kind="ExternalOutput")

    # DRAM scratch (internal)
    senc_d = nc.dram_tensor("senc_scr", [A, BL, H], f16)
    vals_d = nc.dram_tensor("vals_scr", [A, BL, H], f16)      # attended values, [a,b,h]
    vraw_d = nc.dram_tensor("vraw_scr", [NCH, P, A * H], f16)  # V proj [c, b, (j,k,d)]
    w_scr = nc.dram_tensor("w_scr", [NCH, P, A * HEADS * A], f16)  # [c, b, (j,k,i)]
    mask_scr = nc.dram_tensor("mask_scr", [16, 16], f16)

    with tile.TileContext(nc) as tc, ExitStack() as ctx:
        const = ctx.enter_context(tc.tile_pool(name="const", bufs=1))
        wpool = ctx.enter_context(tc.tile_pool(name="weights", bufs=1))
        io = ctx.enter_context(tc.tile_pool(name="io", bufs=2))
        scr = ctx.enter_context(tc.tile_pool(name="scr", bufs=2))
        work = ctx.enter_context(tc.tile_pool(name="work", bufs=3))
        chunkp = ctx.enter_context(tc.tile_pool(name="chunk", bufs=2))
        bpool = ctx.enter_context(tc.tile_pool(name="phaseb", bufs=2))
        fatp = ctx.enter_context(tc.tile_pool(name="fat", bufs=2))
        ps = ctx.enter_context(tc.tile_pool(name="ps", bufs=2, space="PSUM"))
        psg_pool = ctx.enter_context(tc.tile_pool(name="psg", bufs=2, space="PSUM"))

        ident = const.tile([P, P], f16)
        make_identity(nc, ident[:])
        eps_t = const.tile([P, 1], f32)
        nc.vector.memset(eps_t[:], EPS)
        # block-diag mask [(b16,j),(b16',i)] = 1 iff b16==b16', via DRAM
        # bounce of ident16 with broadcast APs
        nc.sync.dma_start(mask_scr[:], ident[0:16, 0:16])
        mask = const.tile([P, P], f16)
        for b16r in range(16):
            nc.sync.dma_start(
                mask[8 * b16r : 8 * b16r + 8, :].rearrange("p (i b) -> p i b", i=8),
                bass.AP(mask_scr, b16r * 16, [[0, 8], [0, 8], [1, 16]]),
            )

        # ---- resident weights ----
        w_ss = wpool.tile([P, A * H], f16, tag="w_ss")
        nc.sync.dma_start(w_ss[:].rearrange("p (a h) -> p a h", a=A), ws_s_d[:].rearrange("a s h -> s a h"))
        w_ssa = wpool.tile([P, A * H], f16, tag="w_ssa")
        nc.sync.dma_start(w_ssa[:].rearrange("p (a h) -> p a h", a=A), ws_sa_d[:, :S, :].rearrange("a s h -> s a h"))
        w_ssa_a = wpool.tile([ADIM, A * H], f16, tag="w_ssa_a")
        nc.sync.dma_start(w_ssa_a[:].rearrange("p (a h) -> p a h", a=A), ws_sa_d[:, S:, :].rearrange("a s h -> s a h"))
        w_k = wpool.tile([P, 4 * H], f16, tag="w_k")
        nc.sync.dma_start(w_k[:].rearrange("p (t n) -> p t n", t=4), wk_d[:].rearrange("(t p) n -> p t n", p=P))
        w_v = wpool.tile([P, 4 * H], f16, tag="w_v")
        nc.sync.dma_start(w_v[:].rearrange("p (t n) -> p t n", t=4), wv_d[:].rearrange("(t p) n -> p t n", p=P))
        w_sel = wpool.tile([P, 4 * H], f16, tag="w_sel")
        nc.sync.dma_start(w_sel[:].rearrange("p (t n) -> p t n", t=4), wsel_d[:].rearrange("(t p) n -> p t n", p=P))

        if nonzero_bias:
            bias_ss = wpool.tile([P, A * H], f16, tag="b_ss")
            nc.sync.dma_start(bias_ss[:], _bc_front(bs_s_d[:].rearrange("a h -> (a h)"), P))
            bias_ssa = wpool.tile([P, A * H], f16, tag="b_ssa")
            nc.sync.dma_start(bias_ssa[:], _bc_front(bs_sa_d[:].rearrange("a h -> (a h)"), P))
            bias_sel = wpool.tile([P, H], f16, tag="b_sel")
            nc.sync.dma_start(bias_sel[:], _bc_front(bsel_d[:], P))
        bias_c1 = wpool.tile([P, A * 4], f32, tag="b_c1")  # feature-major [128h x (a,ht)]
        nc.sync.dma_start(
            bias_c1[:].rearrange("p (a ht) -> p a ht", a=A),
            bc1_d[:].rearrange("a (ht p) -> p a ht", p=P),
        )
        bias_c2 = wpool.tile([ADIM, A], f32, tag="b_c2")
        nc.sync.dma_start(bias_c2[:], bc2_d[:].rearrange("a o -> o a"))

        inv_s = 1.0 / S
        inv_sa = 1.0 / SA
        inv_2h = 1.0 / (2 * H)
        sxA = wpool.tile([P, A * NCH], f32, tag="sxA")  # sum_h s_enc per (a, chunk)

        def emit_phase_b(sc):
            for a in range(A):
                wc1 = scr.tile([P, 8 * H], f16, tag="scaled")  # [128f x 8ft, 512h]
                nc.sync.dma_start(
                wc1[:].rearrange("p (ft h) -> p ft h", ft=8),
                wc1_d[a, :, :].rearrange("(ft p) h -> p ft h", p=P),
                )
                wc2 = scr.tile([P, 4 * ADIM], f16, tag="t2")  # [128h x 4ht, 16]
                nc.sync.dma_start(
                wc2[:].rearrange("p (ht o) -> p ht o", ht=4),
                wc2_d[a, :, :].rearrange("(ht p) o -> p ht o", p=P),
                )
                wc13 = wc1[:].rearrange("p (ft h) -> p ft h", ft=8)
                wc23 = wc2[:].rearrange("p (ht o) -> p ht o", ht=4)
                sb0 = sc * 512
                se_b = bpool.tile([P, 4 * H], f16, tag="se_b")  # [128, 4c, 512]
                nc.sync.dma_start(
                    se_b[:].rearrange("p (c h) -> p c h", c=4),
                    senc_d[a, sb0 : sb0 + 512, :].rearrange("(c p) h -> p c h", p=P),
                )
                se3 = se_b[:].rearrange("p (c h) -> p c h", c=4)
                va_b = bpool.tile([P, 4 * H], f16, tag="va_b")
                nc.sync.dma_start(
                    va_b[:].rearrange("p (c h) -> p c h", c=4),
                    vals_d[a, sb0 : sb0 + 512, :].rearrange(
                        "(c p) h -> p c h", p=P
                    ),
                )
                va3 = va_b[:].rearrange("p (c h) -> p c h", c=4)
                # cin LN stats per (b-row): [128, 4c]
                sx = work.tile([P, 4], f32, tag="sx_cin")
                sx2 = work.tile([P, 4], f32, tag="sx2_cin")
                tmp = work.tile([P, 4], f32, tag="tmp_cin")
                nc.vector.tensor_reduce(out=tmp[:], in_=va3, axis=AX.X, op=FP.add)
                nc.vector.tensor_tensor(
                    out=sx[:],
                    in0=sxA[:, a * NCH + sc * 4 : a * NCH + sc * 4 + 4],
                    in1=tmp[:],
                    op=FP.add,
                )
                sq_scr2 = work.tile([P, H], f32, tag="sq_scr2")
                for cc in range(4):
                    nc.scalar.activation(
                        sq_scr2[:], se3[:, cc, :], AF.Square,
                        accum_out=sx2[:, cc : cc + 1],
                    )
                    nc.scalar.activation(
                        sq_scr2[:], va3[:, cc, :], AF.Square,
                        accum_out=tmp[:, cc : cc + 1],
                    )
                nc.vector.tensor_tensor(out=sx2[:], in0=sx2[:], in1=tmp[:], op=FP.add)
                mean = work.tile([P, 4], f32, tag="mean_cin")
                var = work.tile([P, 4], f32, tag="var_cin")
                msq = work.tile([P, 4], f32, tag="msq_cin")
                rstd = work.tile([P, 4], f32, tag="rstd_cin")
                nc.vector.tensor_scalar_mul(mean[:], sx[:], inv_2h)
                nc.vector.tensor_scalar_mul(var[:], sx2[:], inv_2h)
                nc.vector.tensor_tensor(out=msq[:], in0=mean[:], in1=mean[:], op=FP.mult)
                nc.vector.tensor_tensor(out=var[:], in0=var[:], in1=msq[:], op=FP.subtract)
                nc.scalar.activation(msq[:], var[:], AF.Ln, bias=eps_t[:])
                nc.scalar.activation(rstd[:], msq[:], AF.Exp, scale=-0.5)

                # normalize + transpose -> cinT [128f, 8ft, 512b] fp16
                cinT = bpool.tile([P, 8 * 512], f16, tag="cinT")
                cinT3 = cinT[:].rearrange("p (ft b) -> p ft b", ft=8)
                nrm = work.tile([P, H], f16, tag="nrm")
                for cc in range(4):
                    for half, src3 in ((0, se3), (1, va3)):
                        nc.vector.tensor_scalar(
                            out=nrm[:],
                            in0=src3[:, cc, :],
                            scalar1=mean[:, cc : cc + 1],
                            scalar2=rstd[:, cc : cc + 1],
                            op0=FP.subtract,
                            op1=FP.mult,
                        )
                        ps_c = ps.tile([P, H], f16, tag="tr")
                        for t in range(4):
                            nc.tensor.transpose(
                                ps_c[:, t * P : (t + 1) * P],
                                nrm[:, t * P : (t + 1) * P],
                                ident[:],
                            )
                        nc.scalar.copy(
                            cinT3[:, half * 4 : half * 4 + 4, cc * P : (cc + 1) * P],
                            ps_c[:].rearrange("p (t b) -> p t b", t=4),
                        )

                # mm1: h1_T [128h x 4ht, 512b] = Wc1.T @ cinT
                h1T = bpool.tile([P, 4 * 512], f16, tag="h1T")
                h1T3 = h1T[:].rearrange("p (ht b) -> p ht b", ht=4)
                for ht in range(4):
                    ps_h1 = ps.tile([P, 512], f32, tag="mm")
                    for ft in range(8):
                        nc.tensor.matmul(
                            ps_h1[:],
                            wc13[:, ft, ht * P : (ht + 1) * P],
                            cinT3[:, ft, :],
                            start=(ft == 0),
                            stop=(ft == 7),
                        )
                    nc.scalar.activation(
                        h1T3[:, ht, :], ps_h1[:], AF.Prelu,
                        bias=bias_c1[:, a * 4 + ht : a * 4 + ht + 1],
                        alpha=LRELU_SLOPE,
                    )
                # mm2: allqs_T [16, 512b]
                ps_q = ps.tile([ADIM, 512], f32, tag="mm")
                for ht in range(4):
                    nc.tensor.matmul(
                        ps_q[:],
                        wc23[:, ht, :],
                        h1T3[:, ht, :],
                        start=(ht == 0),
                        stop=(ht == 3),
                    )
                qs = work.tile([ADIM, 512], f32, tag="qs")
                nc.scalar.activation(
                    qs[:], ps_q[:], AF.Identity, bias=bias_c2[:, a : a + 1]
                )
                nc.sync.dma_start(allqs_d[a, :, sb0 : sb0 + 512], qs[:])

        # ================= PHASE A =================
        for c in range(NCH):
            b0 = c * P
            st_t = [None] * A
            ac_t = [None] * A
            sx_st = chunkp.tile([P, A], f32, tag="sx_st")
            sq_st = chunkp.tile([P, A], f32, tag="sq_st")
            sx_sa = chunkp.tile([P, A], f32, tag="sx_sa")
            sq_sa = chunkp.tile([P, A], f32, tag="sq_sa")
            sq_scr = chunkp.tile([P, S], f32, tag="sq_scr")
            for a in range(A):
                st = io.tile([P, S], f32, tag=f"st{a}")
                nc.sync.dma_start(st[:], st_d[a, b0 : b0 + P, :])
                ac = io.tile([P, ADIM], f32, tag=f"ac{a}")
                nc.sync.dma_start(ac[:], ac_d[a, b0 : b0 + P, :])
                st_t[a], ac_t[a] = st, ac
                nc.vector.tensor_reduce(
                    out=sx_st[:, a : a + 1], in_=st[:], axis=AX.X, op=FP.add
                )
                nc.scalar.activation(
                    sq_scr[:], st[:], AF.Square, accum_out=sq_st[:, a : a + 1]
                )
                nc.vector.tensor_reduce(
                    out=sx_sa[:, a : a + 1], in_=ac[:], axis=AX.X, op=FP.add
                )
                nc.scalar.activation(
                    sq_scr[:, :ADIM], ac[:], AF.Square, accum_out=sq_sa[:, a : a + 1]
                )
            nc.vector.tensor_tensor(out=sx_sa[:], in0=sx_sa[:], in1=sx_st[:], op=FP.add)
            nc.vector.tensor_tensor(out=sq_sa[:], in0=sq_sa[:], in1=sq_st[:], op=FP.add)

            def ln_scalars(sx, sq, inv_n, tag):
                mean = chunkp.tile([P, A], f32, tag=f"mean_{tag}")
                rstd = chunkp.tile([P, A], f32, tag=f"rstd_{tag}")
                var = chunkp.tile([P, A], f32, tag=f"var_{tag}")
                nc.vector.tensor_scalar_mul(mean[:], sx[:], inv_n)
                nc.vector.tensor_scalar_mul(var[:], sq[:], inv_n)
                msq = chunkp.tile([P, A], f32, tag=f"msq_{tag}")
                nc.vector.tensor_tensor(out=msq[:], in0=mean[:], in1=mean[:], op=FP.mult)
                nc.vector.tensor_tensor(out=var[:], in0=var[:], in1=msq[:], op=FP.subtract)
                lnv = chunkp.tile([P, A], f32, tag=f"lnv_{tag}")
                nc.scalar.activation(lnv[:], var[:], AF.Ln, bias=eps_t[:])
                nc.scalar.activation(rstd[:], lnv[:], AF.Exp, scale=-0.5)
                return mean, rstd

            mean_st, rstd_st = ln_scalars(sx_st, sq_st, inv_s, "st")
            mean_sa, rstd_sa = ln_scalars(sx_sa, sq_sa, inv_sa, "sa")

            # per-chunk shared attention inputs (batch-major)
            keys_all = chunkp.tile([P, A * H], f16, tag="keys_all")
            vals_all = chunkp.tile([P, A * H], f16, tag="vals_all")
            sel_all = chunkp.tile([P, A * H], f16, tag="sel_all")

            for a in range(A):
                st, ac = st_t[a], ac_t[a]
                stn = work.tile([P, S], f16, tag="stn")
                nc.vector.tensor_scalar(
                    out=stn[:],
                    in0=st[:],
                    scalar1=mean_st[:, a : a + 1],
                    scalar2=rstd_st[:, a : a + 1],
                    op0=FP.subtract,
                    op1=FP.mult,
                )
                san = work.tile([P, SA], f16, tag="san")
                nc.vector.tensor_scalar(
                    out=san[:, :S],
                    in0=st[:],
                    scalar1=mean_sa[:, a : a + 1],
                    scalar2=rstd_sa[:, a : a + 1],
                    op0=FP.subtract,
                    op1=FP.mult,
                )
                nc.vector.tensor_scalar(
                    out=san[:, S:],
                    in0=ac[:],
                    scalar1=mean_sa[:, a : a + 1],
                    scalar2=rstd_sa[:, a : a + 1],
                    op0=FP.subtract,
                    op1=FP.mult,
                )
                ps_t = ps.tile([P, S + SA], f16, tag="tr")
                nc.tensor.transpose(ps_t[:, :S], stn[:], ident[:])
                nc.tensor.transpose(ps_t[:, S : 2 * S], san[:, :S], ident[:])
                ps_ta = ps.tile([ADIM, P], f16, tag="tra")
                nc.tensor.transpose(ps_ta[:], san[:, S:], ident[:])
                stnT = work.tile([P, S], f16, tag="stnT")
                sanT = work.tile([P, S], f16, tag="sanT")
                sanTa = work.tile([ADIM, P], f16, tag="sanTa")
                nc.scalar.copy(stnT[:], ps_t[:, :S])
                nc.scalar.copy(sanT[:], ps_t[:, S : 2 * S])
                nc.scalar.copy(sanTa[:], ps_ta[:])

                # s_enc = lrelu(stn @ Ws_s[a]) : [128b, 512]
                ps_se = ps.tile([P, H], f32, tag="mm")
                nc.tensor.matmul(
                    ps_se[:], stnT[:], w_ss[:, a * H : (a + 1) * H], start=True, stop=True
                )
                senc = work.tile([P, H], f16, tag="senc")
                if nonzero_bias:
                    tmp = work.tile([P, H], f32, tag="senc_tmp")
                    nc.vector.tensor_tensor(
                        out=tmp[:], in0=ps_se[:], in1=bias_ss[:, a * H : (a + 1) * H], op=FP.add
                    )
                    nc.scalar.activation(
                        senc[:], tmp[:], AF.Prelu, alpha=LRELU_SLOPE,
                        accum_out=sxA[:, a * NCH + c : a * NCH + c + 1],
                    )
                else:
                    nc.scalar.activation(
                        senc[:], ps_se[:], AF.Prelu, alpha=LRELU_SLOPE,
                        accum_out=sxA[:, a * NCH + c : a * NCH + c + 1],
                    )
                nc.sync.dma_start(senc_d[a, b0 : b0 + P, :], senc[:])

                # sa_enc = lrelu(san @ Ws_sa[a]) : [128b, 512]
                ps_sa = ps.tile([P, H], f32, tag="mm")
                nc.tensor.matmul(
                    ps_sa[:], sanT[:], w_ssa[:, a * H : (a + 1) * H], start=True, stop=False
                )
                nc.tensor.matmul(
                    ps_sa[:], sanTa[:], w_ssa_a[:, a * H : (a + 1) * H], start=False, stop=True
                )
                saenc = work.tile([P, H], f16, tag="saenc")
                if nonzero_bias:
                    tmp2 = work.tile([P, H], f32, tag="saenc_tmp")
                    nc.vector.tensor_tensor(
                        out=tmp2[:], in0=ps_sa[:], in1=bias_ssa[:, a * H : (a + 1) * H], op=FP.add
                    )
                    nc.scalar.activation(saenc[:], tmp2[:], AF.Prelu, alpha=LRELU_SLOPE)
                else:
                    nc.scalar.activation(saenc[:], ps_sa[:], AF.Prelu, alpha=LRELU_SLOPE)

                # transpose sa_enc -> [512h, 128b]
                ps_saT = ps.tile([P, H], f16, tag="tr")
                for t in range(4):
                    nc.tensor.transpose(
                        ps_saT[:, t * P : (t + 1) * P],
                        saenc[:, t * P : (t + 1) * P],
                        ident[:],
                    )
                saencT = work.tile([P, 4 * P], f16, tag="saencT")  # [128h x 4ht, 128b]
                nc.scalar.copy(saencT[:], ps_saT[:])
                saencT3 = saencT[:].rearrange("p (t b) -> p t b", t=4)

                # keys/vals/sel: [128b, 512(head,d)]
                for (wt, dst, act) in (
                    (w_k, keys_all, False),
                    (w_v, vals_all, False),
                    (w_sel, sel_all, True),
                ):
                    ps_kv = ps.tile([P, H], f32, tag="mm")
                    wt3 = wt[:].rearrange("p (t n) -> p t n", t=4)
                    for t in range(4):
                        nc.tensor.matmul(
                            ps_kv[:],
                            saencT3[:, t, :],
                            wt3[:, t, :],
                            start=(t == 0),
                            stop=(t == 3),
                        )
                    dslice = dst[:, a * H : (a + 1) * H]
                    if act:
                        if nonzero_bias:
                            tmp3 = work.tile([P, H], f32, tag="sel_tmp")
                            nc.vector.tensor_tensor(
                                out=tmp3[:], in0=ps_kv[:], in1=bias_sel[:], op=FP.add
                            )
                            nc.scalar.activation(dslice, tmp3[:], AF.Prelu, alpha=LRELU_SLOPE)
                        else:
                            nc.scalar.activation(dslice, ps_kv[:], AF.Prelu, alpha=LRELU_SLOPE)
                    else:
                        nc.scalar.copy(dslice, ps_kv[:])

            # V proj to DRAM for the fat shuffle
            nc.sync.dma_start(vraw_d[c], vals_all[:])

            # ---- attention middle ----
            # logits via fused TTR: e_st [128b, (j,k,i)] fp32
            e_st = fatp.tile([P, A * HEADS * A], f32, tag="e_st")
            prod_scr = fatp.tile([P, D], f16, tag="prod")
            keys4 = keys_all[:].rearrange("p (j k d) -> p j k d", j=A, k=HEADS)
            sel4 = sel_all[:].rearrange("p (i k d) -> p i k d", i=A, k=HEADS)
            e4 = e_st[:].rearrange("p (j k i) -> p j k i", j=A, k=HEADS)
            for k in range(HEADS):
                for i in range(A):
                    for j in range(A):
                        if j == i:
                            continue
                        nc.vector.affine_mul_reduce(
                            out=prod_scr[:],
                            accum_out=e4[:, j, k, i : i + 1],
                            in0=sel4[:, i, k, :],
                            in1=keys4[:, j, k, :],
                            scale=1.0,
                            bias=0.0,
                        )
            # diag (j==i) never written by TTR; zero it so exp input is defined
            diag32 = bass.AP(
                e_st.tensor, e_st[:].offset,
                [list(e_st[:].ap[0]), [(HEADS * A + 1), A], [A, HEADS]],
            )
            nc.vector.memset(diag32, 0.0)
            # e16 = exp(logits/sqrt(D)) fp16; zero the diagonal (j==i)
            e16 = fatp.tile([P, A * HEADS * A], f16, tag="e16")
            nc.scalar.activation(e16[:], e_st[:], AF.Exp, scale=INV_SQRT_D)
            diag = bass.AP(
                e16.tensor, e16[:].offset,
                [list(e16[:].ap[0]), [(HEADS * A + 1), A], [A, HEADS]],
            )
            nc.vector.memset(diag, 0.0)
            # z[b, (k,i)] = sum_j e16; rz = 1/z
            z_t = fatp.tile([P, HEADS * A], f32, tag="z_t")
            e_for_z = bass.AP(
                e16.tensor, e16[:].offset,
                [list(e16[:].ap[0]), [1, HEADS * A], [HEADS * A, A]],
            )
            nc.vector.tensor_reduce(out=z_t[:], in_=e_for_z, axis=AX.X, op=FP.add)
            rz = fatp.tile([P, HEADS * A], f32, tag="rz")
            nc.vector.reciprocal(rz[:], z_t[:])
            # w16[b, (j,k,i)] = e16 * rz (broadcast over j)
            w16 = fatp.tile([P, A * HEADS * A], f16, tag="w16")
            nc.vector.tensor_tensor(
                out=w16[:].rearrange("p (j n) -> p j n", j=A),
                in0=e16[:].rearrange("p (j n) -> p j n", j=A),
                in1=_bc(rz[:], 0, A),
                op=FP.mult,
            )
            # w16 -> DRAM -> wp [(b16,j), (g,k,i)]
            nc.sync.dma_start(w_scr[c], w16[:])
            wp = fatp.tile([P, 8 * HEADS * A], f16, tag="wp")
            src_w = w_scr[c].rearrange("(g b16) (j n) -> (b16 j) g n", g=8, j=8)
            nc.sync.dma_start(wp[:].rearrange("p (g n) -> p g n", g=8), src_w)
            # vals -> fat [(b16,j), (g,k,d)] in two 4-group halves
            wp3 = wp[:].rearrange("p (g k i) -> p g k i", g=8, k=HEADS)
            src_v_full = vraw_d[c].rearrange(
                "(g b16) (j n) -> (b16 j) g n", g=8, j=8
            )
            for hh in range(2):
                vfat = fatp.tile([P, 4 * H], f16, tag="vfat")
                nc.sync.dma_start(
                    vfat[:].rearrange("p (g n) -> p g n", g=4),
                    src_v_full[:, 4 * hh : 4 * hh + 4, :],
                )
                vfat3 = vfat[:].rearrange("p (g k d) -> p g k d", g=4, k=HEADS)
                vout = fatp.tile([P, 4 * H], f16, tag="vout")
                vout3 = vout[:].rearrange("p (g k d) -> p g k d", g=4, k=HEADS)
                for g4 in range(4):
                    g = 4 * hh + g4
                    psgt = psg_pool.tile([P, H], f32, tag="mm")
                    for k in range(HEADS):
                        wst = work.tile([P, P], f16, tag="wst")
                        wpap = wp3[:, g, k, :]
                        wdims = [list(d) for d in wpap.ap]
                        wdims.append([0, 16])  # bcast b16' innermost (after i)
                        wp_bc = bass.AP(wp.tensor, wpap.offset, wdims)
                        nc.gpsimd.tensor_tensor(
                            out=wst[:], in0=mask[:], in1=wp_bc, op=FP.mult,
                        )
                        nc.tensor.matmul(
                            psgt[:, k * D : (k + 1) * D], wst[:],
                            vfat3[:, g4, k, :],
                            start=True, stop=True,
                        )
                    nc.scalar.copy(
                        vout3[:, g4],
                        psgt[:].rearrange("p (k d) -> p k d", k=HEADS),
                    )
                # vout [(i,b16), (g,k,d)] -> vals_d[a=i, b, h] batch-major
                for i in range(A):
                    dst_i = bass.AP(
                        vals_d, i * BL * H + c * P * H + 4 * hh * 16 * H,
                        [[H, 16],            # b16
                         [16 * H, 4],        # g
                         [1, H]],            # (k,d)
                    )
                    src_i = vout[16 * i : 16 * i + 16, :].rearrange(
                        "p (g n) -> p g n", g=4
                    )
                    nc.sync.dma_start(dst_i, src_i)

            if c == NCH // 2 - 1:
                emit_phase_b(0)
            elif c == NCH - 1:
                emit_phase_b(1)



    nc.compile()
    return nc


_CACHE = {}


def kernel(**inputs):
    states = np.asarray(inputs["states"], np.float32)
    actions = np.asarray(inputs["actions"], np.float32)
    Ws_s = np.asarray(inputs["Ws_s"], np.float32)
    bs_s = np.asarray(inputs["bs_s"], np.float32)
    Ws_sa = np.asarray(inputs["Ws_sa"], np.float32)
    bs_sa = np.asarray(inputs["bs_sa"], np.float32)
    Wk = np.asarray(inputs["Wk"], np.float32)
    Wv = np.asarray(inputs["Wv"], np.float32)
    Wsel = np.asarray(inputs["Wsel"], np.float32)
    bsel = np.asarray(inputs["bsel"], np.float32)
    Wc1 = np.asarray(inputs["Wc1"], np.float32)
    bc1 = np.asarray(inputs["bc1"], np.float32)
    Wc2 = np.asarray(inputs["Wc2"], np.float32)
    bc2 = np.asarray(inputs["bc2"], np.float32)

    nonzero_bias = bool(
        np.any(bs_s) or np.any(bs_sa) or np.any(bsel)
    )
    if nonzero_bias not in _CACHE:
        _CACHE[nonzero_bias] = build(nonzero_bias)
    nc = _CACHE[nonzero_bias]

    wk_f = np.ascontiguousarray(Wk.transpose(1, 0, 2).reshape(H, H)).astype(np.float16)
    wv_f = np.ascontiguousarray(Wv.transpose(1, 0, 2).reshape(H, H)).astype(np.float16)
    wsel_f = np.ascontiguousarray(Wsel.transpose(1, 0, 2).reshape(H, H)).astype(np.float16)
    shared = {
        "ws_s": Ws_s.astype(np.float16),
        "ws_sa": Ws_sa.astype(np.float16),
        "wk": wk_f,
        "wv": wv_f,
        "wsel": wsel_f,
        "wc1": Wc1.astype(np.float16),
        "wc2": Wc2.astype(np.float16),
        "bs_s": bs_s.astype(np.float16),
        "bs_sa": bs_sa.astype(np.float16),
        "bsel": bsel.astype(np.float16),
        "bc1": bc1,
        "bc2": bc2,
    }
    in_maps = []
    for core in range(NCORES):
        sl = slice(core * BL, (core + 1) * BL)
        m = dict(shared)
        m["states"] = np.ascontiguousarray(states[:, sl, :])
        m["actions"] = np.ascontiguousarray(actions[:, sl, :])
        in_maps.append(m)

    res = run_bass_kernel_spmd(nc, in_maps, core_ids=list(range(NCORES)))
    idx = np.argmax(actions, axis=-1)  # [A, B]
    q = np.empty((A, B, 1), np.float32)
    for core in range(NCORES):
        allqs = res.results[core]["allqs"]  # [A, ADIM, BL]
        sl = slice(core * BL, (core + 1) * BL)
        ii = idx[:, sl]
        q[:, sl, 0] = np.take_along_axis(
            allqs, ii[:, None, :], axis=1
        )[:, 0, :]
    return q


def _install_ntff_hook():
    """The agent image's antenv lacks axon_hooks; synthesize it so
    run_bass_kernel_spmd(trace=True) can capture NTFF profiles."""
    import types
    import antenv

    if "antenv.axon_hooks" in sys.modules:
        return
    mod = types.ModuleType("antenv.axon_hooks")
    mod._hook = None

    def set_axon_ntff_profile_hook(h):
        mod._hook = h

    def get_axon_ntff_profile_hook():
        return mod._hook

    mod.set_axon_ntff_profile_hook = set_axon_ntff_profile_hook
    mod.get_axon_ntff_profile_hook = get_axon_ntff_profile_hook
    sys.modules["antenv.axon_hooks"] = mod
    antenv.axon_hooks = mod
    sys.path.insert(0, "/root/.axon_site")
    from trn_agent_boot.trn_boot import _ntff_profile_via_ctypes

    hook = _ntff_profile_via_ctypes("/opt/axon/libaxon_pjrt.so")
    if hook is not None:
        set_axon_ntff_profile_hook(hook)


def profile_run(inputs):
    """Traced run returning HW exec time in ns (max across cores)."""
    import os
    os.environ["BASS_PERFETTO_PROFILE_ALL_CORES"] = "1"
    _install_ntff_hook()
    states = np.asarray(inputs["states"], np.float32)
    actions = np.asarray(inputs["actions"], np.float32)
    nonzero_bias = bool(
        np.any(inputs["bs_s"]) or np.any(inputs["bs_sa"]) or np.any(inputs["bsel"])
    )
    if nonzero_bias not in _CACHE:
        _CACHE[nonzero_bias] = build(nonzero_bias)
    nc = _CACHE[nonzero_bias]
    wk_f = np.ascontiguousarray(np.asarray(inputs["Wk"], np.float32).transpose(1, 0, 2).reshape(H, H)).astype(np.float16)
    wv_f = np.ascontiguousarray(np.asarray(inputs["Wv"], np.float32).transpose(1, 0, 2).reshape(H, H)).astype(np.float16)
    wsel_f = np.ascontiguousarray(np.asarray(inputs["Wsel"], np.float32).transpose(1, 0, 2).reshape(H, H)).astype(np.float16)
    shared = {
        "ws_s": np.asarray(inputs["Ws_s"], np.float32).astype(np.float16),
        "ws_sa": np.asarray(inputs["Ws_sa"], np.float32).astype(np.float16),
        "wk": wk_f, "wv": wv_f, "wsel": wsel_f,
        "wc1": np.asarray(inputs["Wc1"], np.float32).astype(np.float16),
        "wc2": np.asarray(inputs["Wc2"], np.float32).astype(np.float16),
        "bs_s": np.asarray(inputs["bs_s"], np.float16),
        "bs_sa": np.asarray(inputs["bs_sa"], np.float16),
        "bsel": np.asarray(inputs["bsel"], np.float16),
        "bc1": np.asarray(inputs["bc1"], np.float32),
        "bc2": np.asarray(inputs["bc2"], np.float32),
    }
    in_maps = []
    for core in range(NCORES):
        sl = slice(core * BL, (core + 1) * BL)
        m = dict(shared)
        m["states"] = np.ascontiguousarray(states[:, sl, :])
        m["actions"] = np.ascontiguousarray(actions[:, sl, :])
        in_maps.append(m)
    res = run_bass_kernel_spmd(
        nc, in_maps, core_ids=list(range(NCORES)), trace=True,
        tmpdir="/tmp/prof", trace_cores=[0],
    )
    print("mean exec:", res.mean_exec_time_ns, "max core:", res.max_exec_time_core_id)
    return res.exec_time_ns


# revision 19
# speedup vs baseline: 1.1995x; 1.1995x over previous
"""AttentionCritic Trainium2 kernel (v2 — PE block-diag attention).

Problem (hardcoded): A=8 agents, B=8192 batch, S=128 state, ADIM=16 act,
H=512 hid, HEADS=4, D=128. 8 NeuronCores, batch-sharded (1024 batch/core).

Pipeline per core (b = 1024 local batch, chunks of 128):
  Phase A (chunk-outer, agent-inner):
    LN(states), LN([states|actions]) in fp32, normalized out in fp16;
    PE-transpose normalized inputs; fp16 matmuls for s_enc / sa_enc
    (batch-major out via activation-as-stationary); PE-transpose sa_enc;
    keys/vals/sel projections (all heads at once).
    Attention middle:
      logits via fused DVE tensor_tensor_reduce (one op per (k,i,j));
      exp/softmax-normalize into w fp16 [b,(j,k,i)];
      w and vals bounce through DRAM into a "fat" layout [(b16,j), ...];
      per (head, 16-batch group) a block-diagonal 128x128 stationary
      (mask * broadcast-w) turns the j-sum into a PE matmul;
      attended values evicted fat and DMA-scattered to DRAM batch-major.
  Phase B (agent-outer, 512-batch superchunks):
    LN(cin) stats in batch-major, normalize, PE-transpose, critic matmuls
    (h1 feature-major so bc1 folds into the Prelu eviction), all_qs out.
  Host: shard/unshard, fp16 weight casts, argmax-gather of all_qs.
"""
import sys

sys.path.insert(0, "/opt/trn_rl_repo")
from contextlib import ExitStack

import numpy as np

import concourse.bass as bass
import concourse.bacc as bacc
import concourse.mybir as mybir
from concourse import tile
from concourse.bass_utils import run_bass_kernel_spmd
from concourse.masks import make_identity

A, B, S, ADIM = 8, 8192, 128, 16
H, HEADS = 512, 4
D = H // HEADS
EPS = 1e-5
NCORES = 8
BL = B // NCORES          # local batch per core = 1024
P = 128                   # partition size
NCH = BL // P             # chunks per core = 8
NSC = BL // 512           # 512-batch superchunks = 2
SA = S + ADIM             # 144

f32 = mybir.dt.float32
f16 = mybir.dt.float16
FP = mybir.AluOpType
AF = mybir.ActivationFunctionType
AX = mybir.AxisListType
LRELU_SLOPE = 0.01
INV_SQRT_D = 1.0 / float(np.sqrt(D))


def _bc(ap, axis, n):
    """Insert a broadcast (step 0, count n) free dim at position `axis`
    (0 = first free dim)."""
    dims = [list(d) for d in ap.ap]
    dims.insert(1 + axis, [0, n])
    return bass.AP(ap.tensor, ap.offset, dims)


def _bc_front(ap, n):
    """Prepend a broadcast dim (for DRAM->SBUF partition replication)."""
    dims = [[0, n]] + [list(d) for d in ap.ap]
    return bass.AP(ap.tensor, ap.offset, dims)


def _patch_act_tables():
    """Restrict bacc's activation-table choices to the one set that covers
    every func we use (exp, ln, square, parametric_relu, identity, copy) so
    no ACT_TABLE_LOAD churn happens mid-kernel."""
    from concourse import hw_specs
    import concourse.bacc as _bacc

    orig = hw_specs.get_activation_tables

    def only_combined(arch):
        t = orig(arch)
        if "natural_log_exp_and_others" not in t:
            return t
        return {
            k: (v if k == "natural_log_exp_and_others" else set())
            for k, v in t.items()
        }

    only_combined.__wrapped__ = orig
    hw_specs.get_activation_tables = only_combined
    _bacc.get_activation_tables = only_combined


import os
_SKIP = set(os.environ.get("KSKIP", "").split(","))


def build(nonzero_bias):
    if "tables" not in _SKIP:
        _patch_act_tables()
    nc = bacc.Bacc("TRN2", target_bir_lowering=False, debug=False)

    # ---- DRAM I/O ----
    st_d = nc.dram_tensor("states", [A, BL, S], f32, kind="ExternalInput")
    ac_d = nc.dram_tensor("actions", [A, BL, ADIM], f32, kind="ExternalInput")
    ws_s_d = nc.dram_tensor("ws_s", [A, S, H], f16, kind="ExternalInput")
    ws_sa_d = nc.dram_tensor("ws_sa", [A, SA, H], f16, kind="ExternalInput")
    wk_d = nc.dram_tensor("wk", [H, H], f16, kind="ExternalInput")   # [h, (head,d)]
    wv_d = nc.dram_tensor("wv", [H, H], f16, kind="ExternalInput")
    wsel_d = nc.dram_tensor("wsel", [H, H], f16, kind="ExternalInput")
    wc1_d = nc.dram_tensor("wc1", [A, 2 * H, H], f16, kind="ExternalInput")
    wc2_d = nc.dram_tensor("wc2", [A, H, ADIM], f16, kind="ExternalInput")
    bs_s_d = nc.dram_tensor("bs_s", [A, H], f16, kind="ExternalInput")
    bs_sa_d = nc.dram_tensor("bs_sa", [A, H], f16, kind="ExternalInput")
    bsel_d = nc.dram_tensor("bsel", [H], f16, kind="ExternalInput")
    bc1_d = nc.dram_tensor("bc1", [A, H], f32, kind="ExternalInput")
    bc2_d = nc.dram_tensor("bc2", [A, ADIM], f32, kind="ExternalInput")

    allqs_d = nc.dram_tensor("allqs", [A, ADIM, BL], f32, kind="ExternalOutput")

    # DRAM scratch (internal)
    senc_d = nc.dram_tensor("senc_scr", [A, BL, H], f16)
    vals_d = nc.dram_tensor("vals_scr", [A, BL, H], f16)      # attended values, [a,b,h]
    vraw_d = nc.dram_tensor("vraw_scr", [NCH, P, A * H], f16)  # V proj [c, b, (j,k,d)]
    w_scr = nc.dram_tensor("w_scr", [NCH, P, A * HEADS * A], f16)  # [c, b, (j,k,i)]
    mask_scr = nc.dram_tensor("mask_scr", [16, 16], f16)

    with tile.TileContext(nc) as tc, ExitStack() as ctx:
        const = ctx.enter_context(tc.tile_pool(name="const", bufs=1))
        wpool = ctx.enter_context(tc.tile_pool(name="weights", bufs=1))
        io = ctx.enter_context(tc.tile_pool(name="io", bufs=2))
        scr = ctx.enter_context(tc.tile_pool(name="scr", bufs=1))
        work = ctx.enter_context(tc.tile_pool(name="work", bufs=3))
        chunkp = ctx.enter_context(tc.tile_pool(name="chunk", bufs=2))
        bpool = ctx.enter_context(tc.tile_pool(name="phaseb", bufs=2))
        satp = ctx.enter_context(tc.tile_pool(name="sat", bufs=1))
        fatp = ctx.enter_context(tc.tile_pool(name="fat", bufs=2))
        ps = ctx.enter_context(tc.tile_pool(name="ps", bufs=2, space="PSUM"))
        psg_pool = ctx.enter_context(tc.tile_pool(name="psg", bufs=2, space="PSUM"))

        ident = const.tile([P, P], f16)
        make_identity(nc, ident[:])
        onecol = const.tile([P, 1], f16)
        nc.vector.memset(onecol[:], 1.0)
        eps_t = const.tile([P, 1], f32)
        nc.vector.memset(eps_t[:], EPS)
        # block-diag mask [(b16,j),(b16',i)] = 1 iff b16==b16', via DRAM
        # bounce of ident16 with broadcast APs
        nc.sync.dma_start(mask_scr[:], ident[0:16, 0:16])
        mask = const.tile([P, P], f16)
        for b16r in range(16):
            nc.sync.dma_start(
                mask[8 * b16r : 8 * b16r + 8, :].rearrange("p (i b) -> p i b", i=8),
                bass.AP(mask_scr, b16r * 16, [[0, 8], [0, 8], [1, 16]]),
            )

        # ---- resident weights ----
        w_ss = wpool.tile([P, A * H], f16, tag="w_ss")
        nc.sync.dma_start(w_ss[:].rearrange("p (a h) -> p a h", a=A), ws_s_d[:].rearrange("a s h -> s a h"))
        w_ssa = wpool.tile([P, A * H], f16, tag="w_ssa")
        nc.sync.dma_start(w_ssa[:].rearrange("p (a h) -> p a h", a=A), ws_sa_d[:, :S, :].rearrange("a s h -> s a h"))
        w_ssa_a = wpool.tile([ADIM, A * H], f16, tag="w_ssa_a")
        nc.sync.dma_start(w_ssa_a[:].rearrange("p (a h) -> p a h", a=A), ws_sa_d[:, S:, :].rearrange("a s h -> s a h"))
        w_k = wpool.tile([P, 4 * H], f16, tag="w_k")
        nc.sync.dma_start(w_k[:].rearrange("p (t n) -> p t n", t=4), wk_d[:].rearrange("(t p) n -> p t n", p=P))
        w_v = wpool.tile([P, 4 * H], f16, tag="w_v")
        nc.sync.dma_start(w_v[:].rearrange("p (t n) -> p t n", t=4), wv_d[:].rearrange("(t p) n -> p t n", p=P))
        w_sel = wpool.tile([P, 4 * H], f16, tag="w_sel")
        nc.sync.dma_start(w_sel[:].rearrange("p (t n) -> p t n", t=4), wsel_d[:].rearrange("(t p) n -> p t n", p=P))

        if nonzero_bias:
            bias_ss = wpool.tile([P, A * H], f16, tag="b_ss")
            nc.sync.dma_start(bias_ss[:], _bc_front(bs_s_d[:].rearrange("a h -> (a h)"), P))
            bias_ssa = wpool.tile([P, A * H], f16, tag="b_ssa")
            nc.sync.dma_start(bias_ssa[:], _bc_front(bs_sa_d[:].rearrange("a h -> (a h)"), P))
            bias_sel_fm = wpool.tile([P, HEADS], f16, tag="b_sel")
            nc.sync.dma_start(
                bias_sel_fm[:].rearrange("p k -> p k"),
                bsel_d[:].rearrange("(k p) -> p k", p=P),
            )
        bias_c1 = wpool.tile([P, A * 4], f32, tag="b_c1")  # feature-major [128h x (a,ht)]
        nc.sync.dma_start(
            bias_c1[:].rearrange("p (a ht) -> p a ht", a=A),
            bc1_d[:].rearrange("a (ht p) -> p a ht", p=P),
        )
        bias_c2 = wpool.tile([ADIM, A], f32, tag="b_c2")
        nc.sync.dma_start(bias_c2[:], bc2_d[:].rearrange("a o -> o a"))

        inv_s = 1.0 / S
        inv_sa = 1.0 / SA
        inv_2h = 1.0 / (2 * H)
        sxA = wpool.tile([P, A * NCH], f32, tag="sxA")  # sum_h s_enc per (a, chunk)

        def emit_phase_b():
          for a in range(A):
            wc1 = scr.tile([P, 8 * H], f16, tag="scaled")  # [128f x 8ft, 512h]
            nc.sync.dma_start(
                wc1[:].rearrange("p (ft h) -> p ft h", ft=8),
                wc1_d[a, :, :].rearrange("(ft p) h -> p ft h", p=P),
            )
            wc2 = scr.tile([P, 4 * ADIM], f16, tag="t2")  # [128h x 4ht, 16]
            nc.sync.dma_start(
                wc2[:].rearrange("p (ht o) -> p ht o", ht=4),
                wc2_d[a, :, :].rearrange("(ht p) o -> p ht o", p=P),
            )
            wc13 = wc1[:].rearrange("p (ft h) -> p ft h", ft=8)
            wc23 = wc2[:].rearrange("p (ht o) -> p ht o", ht=4)
            for sc in range(NSC):
                sb0 = sc * 512
                se_b = bpool.tile([P, 4 * H], f16, tag="se_b")  # [128, 4c, 512]
                nc.sync.dma_start(
                    se_b[:].rearrange("p (c h) -> p c h", c=4),
                    senc_d[a, sb0 : sb0 + 512, :].rearrange("(c p) h -> p c h", p=P),
                )
                se3 = se_b[:].rearrange("p (c h) -> p c h", c=4)
                va_b = bpool.tile([P, 4 * H], f16, tag="va_b")
                nc.sync.dma_start(
                    va_b[:].rearrange("p (c h) -> p c h", c=4),
                    vals_d[a, sb0 : sb0 + 512, :].rearrange(
                        "(c p) h -> p c h", p=P
                    ),
                )
                va3 = va_b[:].rearrange("p (c h) -> p c h", c=4)
                # cin LN stats per (b-row): [128, 4c]
                sx = work.tile([P, 4], f32, tag="sx_cin")
                sx2 = work.tile([P, 4], f32, tag="sx2_cin")
                tmp = work.tile([P, 4], f32, tag="tmp_cin")
                nc.vector.tensor_reduce(out=tmp[:], in_=va3, axis=AX.X, op=FP.add)
                nc.vector.tensor_tensor(
                    out=sx[:],
                    in0=sxA[:, a * NCH + sc * 4 : a * NCH + sc * 4 + 4],
                    in1=tmp[:],
                    op=FP.add,
                )
                sq_scr2 = work.tile([P, H], f32, tag="sq_scr2")
                for cc in range(4):
                    nc.scalar.activation(
                        sq_scr2[:], se3[:, cc, :], AF.Square,
                        accum_out=sx2[:, cc : cc + 1],
                    )
                    nc.scalar.activation(
                        sq_scr2[:], va3[:, cc, :], AF.Square,
                        accum_out=tmp[:, cc : cc + 1],
                    )
                nc.vector.tensor_tensor(out=sx2[:], in0=sx2[:], in1=tmp[:], op=FP.add)
                mean = work.tile([P, 4], f32, tag="mean_cin")
                var = work.tile([P, 4], f32, tag="var_cin")
                msq = work.tile([P, 4], f32, tag="msq_cin")
                rstd = work.tile([P, 4], f32, tag="rstd_cin")
                nc.vector.tensor_scalar_mul(mean[:], sx[:], inv_2h)
                nc.vector.tensor_scalar_mul(var[:], sx2[:], inv_2h)
                nc.vector.tensor_tensor(out=msq[:], in0=mean[:], in1=mean[:], op=FP.mult)
                nc.vector.tensor_tensor(out=var[:], in0=var[:], in1=msq[:], op=FP.subtract)
                nc.scalar.activation(msq[:], var[:], AF.Ln, bias=eps_t[:])
                nc.scalar.activation(rstd[:], msq[:], AF.Exp, scale=-0.5)

                # normalize + transpose -> cinT [128f, 8ft, 512b] fp16
                cinT = bpool.tile([P, 8 * 512], f16, tag="cinT")
                cinT3 = cinT[:].rearrange("p (ft b) -> p ft b", ft=8)
                nrm = work.tile([P, H], f16, tag="nrm")
                for cc in range(4):
                    for half, src3 in ((0, se3), (1, va3)):
                        nc.vector.tensor_scalar(
                            out=nrm[:],
                            in0=src3[:, cc, :],
                            scalar1=mean[:, cc : cc + 1],
                            scalar2=rstd[:, cc : cc + 1],
                            op0=FP.subtract,
                            op1=FP.mult,
                        )
                        ps_c = ps.tile([P, H], f16, tag="tr")
                        for t in range(4):
                            nc.tensor.transpose(
                                ps_c[:, t * P : (t + 1) * P],
                                nrm[:, t * P : (t + 1) * P],
                                ident[:],
                            )
                        nc.scalar.copy(
                            cinT3[:, half * 4 : half * 4 + 4, cc * P : (cc + 1) * P],
                            ps_c[:].rearrange("p (t b) -> p t b", t=4),
                        )

                # mm1: h1_T [128h x 4ht, 512b] = Wc1.T @ cinT
                h1T = bpool.tile([P, 4 * 512], f16, tag="h1T")
                h1T3 = h1T[:].rearrange("p (ht b) -> p ht b", ht=4)
                for ht in range(4):
                    ps_h1 = ps.tile([P, 512], f32, tag="mm")
                    for ft in range(8):
                        nc.tensor.matmul(
                            ps_h1[:],
                            wc13[:, ft, ht * P : (ht + 1) * P],
                            cinT3[:, ft, :],
                            start=(ft == 0),
                            stop=(ft == 7),
                        )
                    nc.scalar.activation(
                        h1T3[:, ht, :], ps_h1[:], AF.Prelu,
                        bias=bias_c1[:, a * 4 + ht : a * 4 + ht + 1],
                        alpha=LRELU_SLOPE,
                    )
                # mm2: allqs_T [16, 512b]
                ps_q = ps.tile([ADIM, 512], f32, tag="mm")
                for ht in range(4):
                    nc.tensor.matmul(
                        ps_q[:],
                        wc23[:, ht, :],
                        h1T3[:, ht, :],
                        start=(ht == 0),
                        stop=(ht == 3),
                    )
                qs = work.tile([ADIM, 512], f32, tag="qs")
                nc.scalar.activation(
                    qs[:], ps_q[:], AF.Identity, bias=bias_c2[:, a : a + 1]
                )
                nc.sync.dma_start(allqs_d[a, :, sb0 : sb0 + 512], qs[:])

        # ================= PHASE A =================
        for c in range(NCH):
            b0 = c * P
            st_t = [None] * A
            ac_t = [None] * A
            sx_st = chunkp.tile([P, A], f32, tag="sx_st")
            sq_st = chunkp.tile([P, A], f32, tag="sq_st")
            sx_sa = chunkp.tile([P, A], f32, tag="sx_sa")
            sq_sa = chunkp.tile([P, A], f32, tag="sq_sa")
            sq_scr = chunkp.tile([P, S], f32, tag="sq_scr")
            for a in range(A):
                st = io.tile([P, S], f32, tag=f"st{a}")
                nc.sync.dma_start(st[:], st_d[a, b0 : b0 + P, :])
                ac = io.tile([P, ADIM], f32, tag=f"ac{a}")
                nc.sync.dma_start(ac[:], ac_d[a, b0 : b0 + P, :])
                st_t[a], ac_t[a] = st, ac
                nc.vector.tensor_reduce(
                    out=sx_st[:, a : a + 1], in_=st[:], axis=AX.X, op=FP.add
                )
                nc.scalar.activation(
                    sq_scr[:], st[:], AF.Square, accum_out=sq_st[:, a : a + 1]
                )
                nc.vector.tensor_reduce(
                    out=sx_sa[:, a : a + 1], in_=ac[:], axis=AX.X, op=FP.add
                )
                nc.scalar.activation(
                    sq_scr[:, :ADIM], ac[:], AF.Square, accum_out=sq_sa[:, a : a + 1]
                )
            nc.vector.tensor_tensor(out=sx_sa[:], in0=sx_sa[:], in1=sx_st[:], op=FP.add)
            nc.vector.tensor_tensor(out=sq_sa[:], in0=sq_sa[:], in1=sq_st[:], op=FP.add)

            def ln_scalars(sx, sq, inv_n, tag):
                mean = chunkp.tile([P, A], f32, tag=f"mean_{tag}")
                rstd = chunkp.tile([P, A], f32, tag=f"rstd_{tag}")
                var = chunkp.tile([P, A], f32, tag=f"var_{tag}")
                nc.vector.tensor_scalar_mul(mean[:], sx[:], inv_n)
                nc.vector.tensor_scalar_mul(var[:], sq[:], inv_n)
                msq = chunkp.tile([P, A], f32, tag=f"msq_{tag}")
                nc.vector.tensor_tensor(out=msq[:], in0=mean[:], in1=mean[:], op=FP.mult)
                nc.vector.tensor_tensor(out=var[:], in0=var[:], in1=msq[:], op=FP.subtract)
                lnv = chunkp.tile([P, A], f32, tag=f"lnv_{tag}")
                nc.scalar.activation(lnv[:], var[:], AF.Ln, bias=eps_t[:])
                nc.scalar.activation(rstd[:], lnv[:], AF.Exp, scale=-0.5)
                return mean, rstd

            mean_st, rstd_st = ln_scalars(sx_st, sq_st, inv_s, "st")
            mean_sa, rstd_sa = ln_scalars(sx_sa, sq_sa, inv_sa, "sa")

            # V stays batch-major; K/sel go feature-major after the agent loop
            vals_all = chunkp.tile([P, A * H], f16, tag="vals_all")
            saencT_all = satp.tile([P, 4 * A * P], f16, tag="saencT_all")
            saT4 = saencT_all[:].rearrange("p (t a b) -> p t a b", t=4, a=A)

            for a in range(A):
                st, ac = st_t[a], ac_t[a]
                stn = work.tile([P, S], f16, tag="stn")
                nc.vector.tensor_scalar(
                    out=stn[:],
                    in0=st[:],
                    scalar1=mean_st[:, a : a + 1],
                    scalar2=rstd_st[:, a : a + 1],
                    op0=FP.subtract,
                    op1=FP.mult,
                )
                san = work.tile([P, SA], f16, tag="san")
                nc.vector.tensor_scalar(
                    out=san[:, :S],
                    in0=st[:],
                    scalar1=mean_sa[:, a : a + 1],
                    scalar2=rstd_sa[:, a : a + 1],
                    op0=FP.subtract,
                    op1=FP.mult,
                )
                nc.vector.tensor_scalar(
                    out=san[:, S:],
                    in0=ac[:],
                    scalar1=mean_sa[:, a : a + 1],
                    scalar2=rstd_sa[:, a : a + 1],
                    op0=FP.subtract,
                    op1=FP.mult,
                )
                ps_t = ps.tile([P, S + SA + P], f16, tag="tr")
                nc.tensor.transpose(ps_t[:, :S], stn[:], ident[:])
                nc.tensor.transpose(ps_t[:, S : 2 * S], san[:, :S], ident[:])
                ps_ta = ps_t[0:ADIM, S + SA : S + SA + P]
                nc.tensor.transpose(ps_ta, san[:, S:], ident[:])
                stnT = work.tile([P, S], f16, tag="stnT")
                sanT = work.tile([P, S], f16, tag="sanT")
                sanTa = work.tile([ADIM, P], f16, tag="sanTa")
                nc.scalar.copy(stnT[:], ps_t[:, :S])
                nc.scalar.copy(sanT[:], ps_t[:, S : 2 * S])
                nc.scalar.copy(sanTa[:], ps_ta)

                # s_enc = lrelu(stn @ Ws_s[a]) : [128b, 512]
                ps_se = ps.tile([P, H], f32, tag="mm")
                nc.tensor.matmul(
                    ps_se[:], stnT[:], w_ss[:, a * H : (a + 1) * H], start=True, stop=True
                )
                senc = work.tile([P, H], f16, tag="senc")
                if nonzero_bias:
                    tmp = work.tile([P, H], f32, tag="senc_tmp")
                    nc.vector.tensor_tensor(
                        out=tmp[:], in0=ps_se[:], in1=bias_ss[:, a * H : (a + 1) * H], op=FP.add
                    )
                    nc.scalar.activation(
                        senc[:], tmp[:], AF.Prelu, alpha=LRELU_SLOPE,
                        accum_out=sxA[:, a * NCH + c : a * NCH + c + 1],
                    )
                else:
                    nc.scalar.activation(
                        senc[:], ps_se[:], AF.Prelu, alpha=LRELU_SLOPE,
                        accum_out=sxA[:, a * NCH + c : a * NCH + c + 1],
                    )
                nc.sync.dma_start(senc_d[a, b0 : b0 + P, :], senc[:])

                # sa_enc = lrelu(san @ Ws_sa[a]) : [128b, 512]
                ps_sa = ps.tile([P, H], f32, tag="mm")
                nc.tensor.matmul(
                    ps_sa[:], sanT[:], w_ssa[:, a * H : (a + 1) * H], start=True, stop=False
                )
                nc.tensor.matmul(
                    ps_sa[:], sanTa[:], w_ssa_a[:, a * H : (a + 1) * H], start=False, stop=True
                )
                saenc = work.tile([P, H], f16, tag="saenc")
                if nonzero_bias:
                    tmp2 = work.tile([P, H], f32, tag="saenc_tmp")
                    nc.vector.tensor_tensor(
                        out=tmp2[:], in0=ps_sa[:], in1=bias_ssa[:, a * H : (a + 1) * H], op=FP.add
                    )
                    nc.scalar.activation(saenc[:], tmp2[:], AF.Prelu, alpha=LRELU_SLOPE)
                else:
                    nc.scalar.activation(saenc[:], ps_sa[:], AF.Prelu, alpha=LRELU_SLOPE)

                # transpose sa_enc -> [512h, 128b]
                ps_saT = ps.tile([P, H], f16, tag="tr")
                for t in range(4):
                    nc.tensor.transpose(
                        ps_saT[:, t * P : (t + 1) * P],
                        saenc[:, t * P : (t + 1) * P],
                        ident[:],
                    )
                nc.scalar.copy(saT4[:, :, a, :], ps_saT[:].rearrange("p (t b) -> p t b", t=4))

                # V projection (batch-major): [128b, 512(head,d)]
                ps_kv = ps.tile([P, H], f32, tag="mm")
                wv3 = w_v[:].rearrange("p (t n) -> p t n", t=4)
                for t in range(4):
                    nc.tensor.matmul(
                        ps_kv[:],
                        saT4[:, t, a, :],
                        wv3[:, t, :],
                        start=(t == 0),
                        stop=(t == 3),
                    )
                nc.scalar.copy(vals_all[:, a * H : (a + 1) * H], ps_kv[:])

            # V proj to DRAM for the fat shuffle
            nc.sync.dma_start(vraw_d[c], vals_all[:])

            # ---- feature-major K / sel projections: [d, (k, a, b)] ----
            keysT_all = chunkp.tile([P, HEADS * A * P], f16, tag="keysT")
            selT_all = chunkp.tile([P, HEADS * A * P], f16, tag="selT")
            keysT4 = keysT_all[:].rearrange("p (k a b) -> p k a b", k=HEADS, a=A)
            selT4 = selT_all[:].rearrange("p (k a b) -> p k a b", k=HEADS, a=A)
            wk3 = w_k[:].rearrange("p (t n) -> p t n", t=4)
            wsel3 = w_sel[:].rearrange("p (t n) -> p t n", t=4)
            for k in range(HEADS):
                for half in range(2):
                    a0 = 4 * half
                    ps_kf = ps.tile([P, H], f32, tag="mm")
                    ps_sf = ps.tile([P, H], f32, tag="mm")
                    for t in range(4):
                        rhs = saT4[:, t, a0 : a0 + 4, :]
                        nc.tensor.matmul(
                            ps_kf[:], wk3[:, t, k * D : (k + 1) * D], rhs,
                            start=(t == 0), stop=(t == 3),
                        )
                        nc.tensor.matmul(
                            ps_sf[:], wsel3[:, t, k * D : (k + 1) * D], rhs,
                            start=(t == 0), stop=(t == 3),
                        )
                    nc.scalar.copy(
                        keysT4[:, k, a0 : a0 + 4, :],
                        ps_kf[:].rearrange("p (a b) -> p a b", a=4),
                    )
                    if nonzero_bias:
                        nc.scalar.activation(
                            selT4[:, k, a0 : a0 + 4, :].rearrange(
                                "p a b -> p (a b)"
                            ),
                            ps_sf[:], AF.Prelu,
                            bias=bias_sel_fm[:, k : k + 1],
                            alpha=LRELU_SLOPE,
                        )
                    else:
                        nc.scalar.activation(
                            selT4[:, k, a0 : a0 + 4, :].rearrange(
                                "p a b -> p (a b)"
                            ),
                            ps_sf[:], AF.Prelu, alpha=LRELU_SLOPE,
                        )

            # ---- logits: DVE broadcast-mul + PE ones-column d-reduce ----
            e_ps = ps.tile([P, HEADS * A * A], f32, tag="eps")
            diag_ps = bass.AP(
                e_ps.tensor, e_ps[:].offset,
                [list(e_ps[:].ap[0]), [(HEADS * A + 1), A], [A, HEADS]],
            )
            nc.vector.memset(diag_ps, 0.0)
            for k in range(HEADS):
                for i in range(A):
                    prod = work.tile([P, A * P], f16, tag="prod")
                    nc.vector.tensor_tensor(
                        out=prod[:].rearrange("p (j b) -> p j b", j=A),
                        in0=_bc(selT4[:, k, i, :], 0, A),
                        in1=keysT4[:, k, :, :],
                        op=FP.mult,
                    )
                    for j in range(A):
                        if j == i:
                            continue
                        col = j * (HEADS * A) + k * A + i
                        nc.tensor.matmul(
                            e_ps[:, col : col + 1],
                            prod[:, j * P : (j + 1) * P],
                            onecol[:],
                            start=True, stop=True,
                        )
            # e16 = exp(logits/sqrt(D)) fp16; zero the diagonal (j==i)
            e16 = fatp.tile([P, A * HEADS * A], f16, tag="e16")
            nc.scalar.activation(e16[:], e_ps[:], AF.Exp, scale=INV_SQRT_D)
            diag = bass.AP(
                e16.tensor, e16[:].offset,
                [list(e16[:].ap[0]), [(HEADS * A + 1), A], [A, HEADS]],
            )
            nc.vector.memset(diag, 0.0)
            # z[b, (k,i)] = sum_j e16; rz = 1/z
            z_t = fatp.tile([P, HEADS * A], f32, tag="z_t")
            e_for_z = bass.AP(
                e16.tensor, e16[:].offset,
                [list(e16[:].ap[0]), [1, HEADS * A], [HEADS * A, A]],
            )
            nc.vector.tensor_reduce(out=z_t[:], in_=e_for_z, axis=AX.X, op=FP.add)
            rz = fatp.tile([P, HEADS * A], f32, tag="rz")
            nc.vector.reciprocal(rz[:], z_t[:])
            # w16[b, (j,k,i)] = e16 * rz (broadcast over j)
            w16 = fatp.tile([P, A * HEADS * A], f16, tag="w16")
            nc.vector.tensor_tensor(
                out=w16[:].rearrange("p (j n) -> p j n", j=A),
                in0=e16[:].rearrange("p (j n) -> p j n", j=A),
                in1=_bc(rz[:], 0, A),
                op=FP.mult,
            )
            # w16 -> DRAM -> wp [(b16,j), (g,k,i)]
            nc.sync.dma_start(w_scr[c], w16[:])
            wp = fatp.tile([P, 8 * HEADS * A], f16, tag="wp")
            src_w = w_scr[c].rearrange("(g b16) (j n) -> (b16 j) g n", g=8, j=8)
            nc.sync.dma_start(wp[:].rearrange("p (g n) -> p g n", g=8), src_w)
            # vals -> fat [(b16,j), (g,k,d)] in two 4-group halves
            wp3 = wp[:].rearrange("p (g k i) -> p g k i", g=8, k=HEADS)
            src_v_full = vraw_d[c].rearrange(
                "(g b16) (j n) -> (b16 j) g n", g=8, j=8
            )
            for hh in range(2):
                vfat = fatp.tile([P, 4 * H], f16, tag="vfat")
                nc.sync.dma_start(
                    vfat[:].rearrange("p (g n) -> p g n", g=4),
                    src_v_full[:, 4 * hh : 4 * hh + 4, :],
                )
                vfat3 = vfat[:].rearrange("p (g k d) -> p g k d", g=4, k=HEADS)
                vout = fatp.tile([P, 4 * H], f16, tag="vout")
                vout3 = vout[:].rearrange("p (g k d) -> p g k d", g=4, k=HEADS)
                for g4 in range(4):
                    g = 4 * hh + g4
                    psgt = psg_pool.tile([P, H], f32, tag="mm")
                    for k in range(HEADS):
                        wst = work.tile([P, P], f16, tag="wst")
                        wpap = wp3[:, g, k, :]
                        wdims = [list(d) for d in wpap.ap]
                        wdims.append([0, 16])  # bcast b16' innermost (after i)
                        wp_bc = bass.AP(wp.tensor, wpap.offset, wdims)
                        nc.vector.tensor_tensor(
                            out=wst[:], in0=mask[:], in1=wp_bc, op=FP.mult,
                        )
                        nc.tensor.matmul(
                            psgt[:, k * D : (k + 1) * D], wst[:],
                            vfat3[:, g4, k, :],
                            start=True, stop=True,
                        )
                    nc.scalar.copy(
                        vout3[:, g4],
                        psgt[:].rearrange("p (k d) -> p k d", k=HEADS),
                    )
                # vout [(i,b16), (g,k,d)] -> vals_d[a=i, b, h] batch-major
                for i in range(A):
                    dst_i = bass.AP(
                        vals_d, i * BL * H + c * P * H + 4 * hh * 16 * H,
                        [[H, 16],            # b16
                         [16 * H, 4],        # g
                         [1, H]],            # (k,d)
                    )
                    src_i = vout[16 * i : 16 * i + 16, :].rearrange(
                        "p (g n) -> p g n", g=4
                    )
                    nc.sync.dma_start(dst_i, src_i)





        emit_phase_b()

    nc.compile()
    return nc


_CACHE = {}


def kernel(**inputs):
    states = np.asarray(inputs["states"], np.float32)
    actions = np.asarray(inputs["actions"], np.float32)
    Ws_s = np.asarray(inputs["Ws_s"], np.float32)
    bs_s = np.asarray(inputs["bs_s"], np.float32)
    Ws_sa = np.asarray(inputs["Ws_sa"], np.float32)
    bs_sa = np.asarray(inputs["bs_sa"], np.float32)
    Wk = np.asarray(inputs["Wk"], np.float32)
    Wv = np.asarray(inputs["Wv"], np.float32)
    Wsel = np.asarray(inputs["Wsel"], np.float32)
    bsel = np.asarray(inputs["bsel"], np.float32)
    Wc1 = np.asarray(inputs["Wc1"], np.float32)
    bc1 = np.asarray(inputs["bc1"], np.float32)
    Wc2 = np.asarray(inputs["Wc2"], np.float32)
    bc2 = np.asarray(inputs["bc2"], np.float32)

    nonzero_bias = bool(
        np.any(bs_s) or np.any(bs_sa) or np.any(bsel)
    )
    if nonzero_bias not in _CACHE:
        _CACHE[nonzero_bias] = build(nonzero_bias)
    nc = _CACHE[nonzero_bias]

    wk_f = np.ascontiguousarray(Wk.transpose(1, 0, 2).reshape(H, H)).astype(np.float16)
    wv_f = np.ascontiguousarray(Wv.transpose(1, 0, 2).reshape(H, H)).astype(np.float16)
    wsel_f = np.ascontiguousarray(Wsel.transpose(1, 0, 2).reshape(H, H)).astype(np.float16)
    shared = {
        "ws_s": Ws_s.astype(np.float16),
        "ws_sa": Ws_sa.astype(np.float16),
        "wk": wk_f,
        "wv": wv_f,
        "wsel": wsel_f,
        "wc1": Wc1.astype(np.float16),
        "wc2": Wc2.astype(np.float16),
        "bs_s": bs_s.astype(np.float16),
        "bs_sa": bs_sa.astype(np.float16),
        "bsel": bsel.astype(np.float16),
        "bc1": bc1,
        "bc2": bc2,
    }
    in_maps = []
    for core in range(NCORES):
        sl = slice(core * BL, (core + 1) * BL)
        m = dict(shared)
        m["states"] = np.ascontiguousarray(states[:, sl, :])
        m["actions"] = np.ascontiguousarray(actions[:, sl, :])
        in_maps.append(m)

    res = run_bass_kernel_spmd(nc, in_maps, core_ids=list(range(NCORES)))
    idx = np.argmax(actions, axis=-1)  # [A, B]
    q = np.empty((A, B, 1), np.float32)
    for core in range(NCORES):
        allqs = res.results[core]["allqs"]  # [A, ADIM, BL]
        sl = slice(core * BL, (core + 1) * BL)
        ii = idx[:, sl]
        q[:, sl, 0] = np.take_along_axis(
            allqs, ii[:, None, :], axis=1
        )[:, 0, :]
    return q


def _install_ntff_hook():
    """The agent image's antenv lacks axon_hooks; synthesize it so
    run_bass_kernel_spmd(trace=True) can capture NTFF profiles."""
    import types
    import antenv

    if "antenv.axon_hooks" in sys.modules:
        return
    mod = types.ModuleType("antenv.axon_hooks")
    mod._hook = None

    def set_axon_ntff_profile_hook(h):
        mod._hook = h

    def get_axon_ntff_profile_hook():
        return mod._hook

    mod.set_axon_ntff_profile_hook = set_axon_ntff_profile_hook
    mod.get_axon_ntff_profile_hook = get_axon_ntff_profile_hook
    sys.modules["antenv.axon_hooks"] = mod
    antenv.axon_hooks = mod
    sys.path.insert(0, "/root/.axon_site")
    from trn_agent_boot.trn_boot import _ntff_profile_via_ctypes

    hook = _ntff_profile_via_ctypes("/opt/axon/libaxon_pjrt.so")
    if hook is not None:
        set_axon_ntff_profile_hook(hook)


def profile_run(inputs):
    """Traced run returning HW exec time in ns (max across cores)."""
    import os
    os.environ["BASS_PERFETTO_PROFILE_ALL_CORES"] = "1"
    _install_ntff_hook()
    states = np.asarray(inputs["states"], np.float32)
    actions = np.asarray(inputs["actions"], np.float32)
    nonzero_bias = bool(
        np.any(inputs["bs_s"]) or np.any(inputs["bs_sa"]) or np.any(inputs["bsel"])
    )
    if nonzero_bias not in _CACHE:
        _CACHE[nonzero_bias] = build(nonzero_bias)
    nc = _CACHE[nonzero_bias]
    wk_f = np.ascontiguousarray(np.asarray(inputs["Wk"], np.float32).transpose(1, 0, 2).reshape(H, H)).astype(np.float16)
    wv_f = np.ascontiguousarray(np.asarray(inputs["Wv"], np.float32).transpose(1, 0, 2).reshape(H, H)).astype(np.float16)
    wsel_f = np.ascontiguousarray(np.asarray(inputs["Wsel"], np.float32).transpose(1, 0, 2).reshape(H, H)).astype(np.float16)
    shared = {
        "ws_s": np.asarray(inputs["Ws_s"], np.float32).astype(np.float16),
        "ws_sa": np.asarray(inputs["Ws_sa"], np.float32).astype(np.float16),
        "wk": wk_f, "wv": wv_f, "wsel": wsel_f,
        "wc1": np.asarray(inputs["Wc1"], np.float32).astype(np.float16),
        "wc2": np.asarray(inputs["Wc2"], np.float32).astype(np.float16),
        "bs_s": np.asarray(inputs["bs_s"], np.float16),
        "bs_sa": np.asarray(inputs["bs_sa"], np.float16),
        "bsel": np.asarray(inputs["bsel"], np.float16),
        "bc1": np.asarray(inputs["bc1"], np.float32),
        "bc2": np.asarray(inputs["bc2"], np.float32),
    }
    in_maps = []
    for core in range(NCORES):
        sl = slice(core * BL, (core + 1) * BL)
        m = dict(shared)
        m["states"] = np.ascontiguousarray(states[:, sl, :])
        m["actions"] = np.ascontiguousarray(actions[:, sl, :])
        in_maps.append(m)
    res = run_bass_kernel_spmd(
        nc, in_maps, core_ids=list(range(NCORES)), trace=True,
        tmpdir="/tmp/prof", trace_cores=[0],
    )
    print("mean exec:", res.mean_exec_time_ns, "max core:", res.max_exec_time_core_id)
    return res.exec_time_ns
